# revision 1
# baseline (speedup 1.0000x reference)
import sys

sys.path.insert(0, "/opt/trn_rl_repo")
import numpy as np
import concourse.bass as bass
import concourse.tile as tile
from concourse import bacc, mybir
from concourse.alu_op_type import AluOpType
from concourse.bass_utils import run_bass_kernel_spmd

# Problem constants (nn_EquivGNNEncoder: 2048 graphs x 32 atoms, 3 layers)
B, NA = 2048, 32
N = B * NA                  # 65536 nodes
S_MUL, V_MUL = 32, 16
NCORES = 8
GPC = B // NCORES           # 256 graphs per core
NPC = GPC * NA              # 8192 nodes per core
GPB = 4                     # graphs per block (4*32 = 128 partitions)
NBLK = GPC // GPB           # 64 blocks per core
LAT = 128                   # latent out dim
HID = 256

INV_SQRT3 = 1.0 / np.sqrt(3.0)
C_SCALAR = np.float32(1.0 / np.sqrt(48.0))
C_VECTOR = np.float32(np.sqrt(3.0 / 48.0))

F32 = mybir.dt.float32
F32R = mybir.dt.float32r
BF16 = mybir.dt.bfloat16

_CACHE = {}

# node feature column layout: [s(0:32) | vx(32:48) | vy(48:64) | vz(64:80)]
# gm column blocks: [mask(0:128) | shx | shy | shz]
# ps_agg rows r = feat col r; repack copies (engine partition bases must be
# 32-aligned, so every copy starts at an aligned src/dst partition):
#   c1: ps_agg[0:80, mask] -> SSa[0:80]    (s_m vx_m vy_m vz_m)
#   c2: ps_agg[0:64, shy ] -> SSb[0:64]    (s_y vx_waste vyy)
#   c3: ps_agg[0:48, shx ] -> SSb[64:112]  (s_x vxx)
#   c4: ps_agg[0:80, shz ] -> SSc[0:80]    (s_z vx_waste vy_waste vzz)


def _build_program():
    nc = bacc.Bacc("TRN2", target_bir_lowering=False, debug=False)

    s0_ap = nc.dram_tensor("s0", [NPC, S_MUL], BF16, kind="ExternalInput").ap()
    posT_ap = nc.dram_tensor("posT", [NBLK, 3, 128], F32, kind="ExternalInput").ap()
    posnm_ap = nc.dram_tensor("posnm", [NPC, 3], F32, kind="ExternalInput").ap()
    bd_ap = nc.dram_tensor("bd", [128, 128], F32, kind="ExternalInput").ap()
    wta_ap = nc.dram_tensor("wta", [3, 80, 80], F32, kind="ExternalInput").ap()
    wtb_ap = nc.dram_tensor("wtb", [3, 112, 80], F32, kind="ExternalInput").ap()
    wtc_ap = nc.dram_tensor("wtc", [3, 80, 80], F32, kind="ExternalInput").ap()
    poolm_ap = nc.dram_tensor("poolm", [128, GPB], F32, kind="ExternalInput").ap()
    wr1_ap = nc.dram_tensor("wr1", [80, HID], F32, kind="ExternalInput").ap()
    br1_ap = nc.dram_tensor("br1", [HID, 1], F32, kind="ExternalInput").ap()
    wr2_ap = nc.dram_tensor("wr2", [HID, LAT], F32, kind="ExternalInput").ap()
    br2_ap = nc.dram_tensor("br2", [LAT, 1], F32, kind="ExternalInput").ap()
    out_ap = nc.dram_tensor("outfm", [LAT, GPC], F32, kind="ExternalOutput").ap()

    with tile.TileContext(nc) as tc:
        with tc.tile_pool(name="const", bufs=1) as const, \
             tc.tile_pool(name="stage", bufs=10) as stage, \
             tc.tile_pool(name="gmp", bufs=9) as gmp, \
             tc.tile_pool(name="featb", bufs=20, space="SBUF") as featbp, \
             tc.tile_pool(name="ssp", bufs=8) as ssp, \
             tc.tile_pool(name="work", bufs=10) as work, \
             tc.tile_pool(name="psagg", bufs=4, space="PSUM") as psp_agg, \
             tc.tile_pool(name="psh", bufs=3, space="PSUM") as psp_h, \
             tc.tile_pool(name="pspool", bufs=1, space="PSUM") as psp_pool:

            # --- constants ---
            bd = const.tile([128, 128], F32)
            nc.sync.dma_start(bd[:], bd_ap[:])
            wta_f = const.tile([80, 3, 80], F32)
            nc.scalar.dma_start(
                wta_f[:],
                bass.AP(tensor=wta_ap.tensor, offset=wta_ap.offset,
                        ap=[[80, 80], [80 * 80, 3], [1, 80]]),
            )
            wta = const.tile([80, 3, 80], BF16)
            nc.vector.tensor_copy(wta[:], wta_f[:])
            wtb_f = const.tile([112, 3, 80], F32)
            nc.scalar.dma_start(
                wtb_f[:],
                bass.AP(tensor=wtb_ap.tensor, offset=wtb_ap.offset,
                        ap=[[80, 112], [112 * 80, 3], [1, 80]]),
            )
            wtb = const.tile([112, 3, 80], BF16)
            nc.vector.tensor_copy(wtb[:], wtb_f[:])
            wtc_f = const.tile([80, 3, 80], F32)
            nc.scalar.dma_start(
                wtc_f[:],
                bass.AP(tensor=wtc_ap.tensor, offset=wtc_ap.offset,
                        ap=[[80, 80], [80 * 80, 3], [1, 80]]),
            )
            wtc = const.tile([80, 3, 80], BF16)
            nc.vector.tensor_copy(wtc[:], wtc_f[:])
            poolm_f = const.tile([128, GPB], F32)
            nc.sync.dma_start(poolm_f[:], poolm_ap[:])
            poolm = const.tile([128, GPB], BF16)
            nc.vector.tensor_copy(poolm[:], poolm_f[:])
            wr1_f = const.tile([80, HID], F32)
            nc.scalar.dma_start(wr1_f[:], wr1_ap[:])
            wr1 = const.tile([80, HID], F32R)
            nc.vector.tensor_copy(wr1[:], wr1_f[:])
            wr2a_f = const.tile([128, LAT], F32)
            nc.scalar.dma_start(wr2a_f[:], wr2_ap[0:128, :])
            wr2a = const.tile([128, LAT], F32R)
            nc.vector.tensor_copy(wr2a[:], wr2a_f[:])
            wr2b_f = const.tile([128, LAT], F32)
            nc.scalar.dma_start(wr2b_f[:], wr2_ap[128:256, :])
            wr2b = const.tile([128, LAT], F32R)
            nc.vector.tensor_copy(wr2b[:], wr2b_f[:])
            br1a = const.tile([128, 1], F32)
            nc.sync.dma_start(br1a[:], br1_ap[0:128, :])
            br1b = const.tile([128, 1], F32)
            nc.sync.dma_start(br1b[:], br1_ap[128:256, :])
            br2 = const.tile([LAT, 1], F32)
            nc.sync.dma_start(br2[:], br2_ap[:])
            epsb = const.tile([128, 1], F32)
            nc.vector.memset(epsb[:], 1e-12)

            # pooled per-graph features, feature-major [80, 256]
            xfm = const.tile([80, GPC], F32R)

            # prime the featb ring: zero cols 80:128 once per pool buffer so
            # the agg lhsT is a full 128-col weight (enables FWL); later
            # writers only touch cols 0:80, so the pad survives rotation
            for _ in range(20):
                t = featbp.tile([128, 128], BF16, tag="fb")
                nc.vector.memset(t[:, 80:128], 0.0)

            def emit_group_stage(bs):
                # batched per-group loads: one DMA each for the group's
                # per-node positions and s0 (DMA dispatch is ~600ns serial
                # on the sync queue regardless of size)
                n = len(bs)
                posg = stage.tile([128, 3 * n], F32, tag="posg")
                nc.sync.dma_start(
                    posg[:],
                    bass.AP(tensor=posnm_ap.tensor,
                            offset=posnm_ap.offset + bs[0] * 128 * 3,
                            ap=[[3, 128], [128 * 3, n], [1, 3]]),
                )
                return posg, None

            def emit_load_gm(b, i, posg, s0g):
                # positions of the 128 in-block atoms, replicated along
                # partitions: f_all[p, 128c+j] = pos[j, c]
                f_all = stage.tile([128, 384], F32, tag="fall")
                nc.sync.dma_start(
                    f_all[:],
                    bass.AP(tensor=posT_ap.tensor, offset=posT_ap.offset + b * 3 * 128,
                            ap=[[0, 128], [128, 3], [1, 128]]),
                )

                # diff[i, 128c+j] = pos[j,c] - pos[i,c]; in1 = per-partition
                # pos broadcast along j via stride-0 free dim
                diff = work.tile([128, 384], F32, tag="diff")
                nc.gpsimd.tensor_sub(
                    diff[:], f_all[:],
                    bass.AP(tensor=posg.tensor, offset=posg.offset + 3 * i,
                            ap=[[posg.shape[1], 128], [1, 3], [0, 128]]))
                sq = work.tile([128, 384], F32, tag="sq")
                nc.scalar.activation(
                    sq[:], diff[:], mybir.ActivationFunctionType.Square)
                # d2 = (dx^2 + dy^2) + dz^2 (exact reference add order)
                d2 = work.tile([128, 128], F32, tag="d2")
                nc.gpsimd.tensor_add(d2[:], sq[:, 0:128], sq[:, 128:256])
                nc.gpsimd.tensor_add(d2[:], d2[:], sq[:, 256:384])

                # mask (block-diag, self-loops excluded in bd), bf16 out
                gm = gmp.tile([128, 512], BF16, tag="gm")
                nc.vector.scalar_tensor_tensor(
                    gm[:, 0:128], d2[:], 25.0, bd[:], AluOpType.is_le, AluOpType.mult)

                # rs = sqrt(3)/|r| ; ga = rs * mask
                s3 = work.tile([128, 128], F32, tag="s3")
                nc.scalar.activation(
                    s3[:], d2[:], mybir.ActivationFunctionType.Sqrt,
                    bias=epsb[:], scale=float(1.0 / 3.0))
                rs = work.tile([128, 128], F32, tag="rs")
                nc.vector.reciprocal_approx_fast(rs[:], s3[:])
                ga = work.tile([128, 128], F32, tag="ga")
                nc.gpsimd.tensor_mul(ga[:], rs[:], gm[:, 0:128])
                # gm_sh[:, 128c+j] = diff_c * ga (ga broadcast over c)
                nc.gpsimd.tensor_mul(
                    gm[:, 128:512], diff[:],
                    bass.AP(tensor=ga.tensor, offset=ga.offset,
                            ap=[[128, 128], [0, 3], [1, 128]]))

                # node features (bf16): s0 in cols 0:32, vectors start at 0,
                # cols 80:128 stay zero (FWL pad, primed at kernel start)
                featb = featbp.tile([128, 128], BF16, tag="fb")
                nc.gpsimd.memset(featb[:, 32:80], 0.0)
                nc.sync.dma_start(featb[:, 0:32], s0_ap[b * 128:(b + 1) * 128, :])
                return gm, featb

            def emit_agg(gm, featb):
                ps_agg = psp_agg.tile([128, 512], F32, tag="agg")
                nc.tensor.matmul(ps_agg[:], featb[:], gm[:], start=True, stop=True)
                return ps_agg

            def emit_copies(ps_agg):
                ssa = ssp.tile([80, 128], BF16, tag="ssa")
                ssb = ssp.tile([112, 128], BF16, tag="ssb")
                ssc = ssp.tile([80, 128], BF16, tag="ssc")
                nc.scalar.copy(ssa[0:80, :], ps_agg[0:80, 0:128])
                nc.scalar.copy(ssb[0:64, :], ps_agg[0:64, 256:384])
                nc.vector.tensor_copy(ssb[64:112, :], ps_agg[0:48, 128:256])
                nc.vector.tensor_copy(ssc[0:80, :], ps_agg[0:80, 384:512])
                return ssa, ssb, ssc

            def emit_transform(l, ss):
                ssa, ssb, ssc = ss
                ps_h = psp_h.tile([128, 80], F32, tag="psh")
                nc.tensor.matmul(ps_h[:], ssa[:], wta[:, l, :], start=True, stop=False)
                nc.tensor.matmul(ps_h[:], ssb[:], wtb[:, l, :], start=False, stop=False)
                nc.tensor.matmul(ps_h[:], ssc[:], wtc[:, l, :], start=False, stop=True)
                return ps_h

            def emit_resid(ps_h, featb):
                featbn = featbp.tile([128, 128], BF16, tag="fb")
                nc.vector.scalar_tensor_tensor(
                    featbn[:, 0:80], ps_h[:], 0.0, featb[:, 0:80],
                    AluOpType.max, AluOpType.add)
                return featbn

            # persistent pool accumulator: every block's pool matmul writes
            # its own 4-column strip; copied to SBUF once at the end
            ps_pool = psp_pool.tile([128, GPC], F32, tag="pool")

            def emit_pool(b, featb):
                nc.tensor.matmul(ps_pool[:, b * GPB:(b + 1) * GPB], featb[:],
                                 poolm[:], start=True, stop=True)

            # interleave independent blocks, phase-ordered so each engine's
            # in-order queue always holds independent work (PE: all aggs
            # queued before any transform that waits on copies)
            group_sizes = [8] * 8
            b0 = 0
            for IW in group_sizes:
                bs = [b0 + i for i in range(IW)]
                b0 += IW
                posg, s0g = emit_group_stage(bs)
                st = [emit_load_gm(bs[i], i, posg, s0g) for i in range(IW)]
                for l in range(3):
                    aggs = [emit_agg(st[i][0], st[i][1]) for i in range(IW)]
                    sss = [emit_copies(aggs[i]) for i in range(IW)]
                    hs = [emit_transform(l, sss[i]) for i in range(IW)]
                    for i in range(IW):
                        st[i] = (st[i][0], emit_resid(hs[i], st[i][1]))
                for i in range(IW):
                    emit_pool(bs[i], st[i][1])

            nc.vector.tensor_copy(xfm[:], ps_pool[0:80, :])

            # --- readout MLP: relu(x @ Wr1 + br1) @ Wr2 + br2, feature-major ---
            ps_h1 = psp_h.tile([128, GPC], F32, tag="psh")
            ps_h2 = psp_h.tile([128, GPC], F32, tag="psh")
            nc.tensor.matmul(ps_h1[:], wr1[:, 0:128], xfm[:], start=True, stop=True)
            nc.tensor.matmul(ps_h2[:], wr1[:, 128:256], xfm[:], start=True, stop=True)
            hid1 = work.tile([128, GPC], F32R, tag="hid1")
            hid2 = work.tile([128, GPC], F32R, tag="hid2")
            nc.vector.tensor_scalar(hid1[:], ps_h1[:], br1a[:], 0.0,
                                    AluOpType.add, AluOpType.max)
            nc.vector.tensor_scalar(hid2[:], ps_h2[:], br1b[:], 0.0,
                                    AluOpType.add, AluOpType.max)
            ps_o = psp_agg.tile([LAT, GPC], F32, tag="agg")
            nc.tensor.matmul(ps_o[:], wr2a[:], hid1[:], start=True, stop=False)
            nc.tensor.matmul(ps_o[:], wr2b[:], hid2[:], start=False, stop=True)
            outt = work.tile([LAT, GPC], F32, tag="outt")
            nc.vector.tensor_scalar(outt[:], ps_o[:], br2[:], None, AluOpType.add)
            nc.sync.dma_start(out_ap[:], outt[:])

    nc.compile()
    return nc


def kernel(pos, emb, W_s2n, W1, W2, W3, W4, Ws, Wv, Wr1, br1, Wr2, br2,
           z, batch, edge_index, num_graphs):
    pos = np.asarray(pos, dtype=np.float32)
    z = np.asarray(z)
    emb = np.asarray(emb, dtype=np.float32)
    W_s2n = np.asarray(W_s2n, dtype=np.float32)
    W1 = np.asarray(W1, dtype=np.float32); W2 = np.asarray(W2, dtype=np.float32)
    W3 = np.asarray(W3, dtype=np.float32); W4 = np.asarray(W4, dtype=np.float32)
    Ws = np.asarray(Ws, dtype=np.float32); Wv = np.asarray(Wv, dtype=np.float32)
    Wr1 = np.asarray(Wr1, dtype=np.float32); br1 = np.asarray(br1, dtype=np.float32)
    Wr2 = np.asarray(Wr2, dtype=np.float32); br2 = np.asarray(br2, dtype=np.float32)

    # host prep: embedding lookup folded with input linear
    import ml_dtypes
    EW = (emb @ W_s2n) * np.float32(1.0 / np.sqrt(S_MUL))     # [100, 32]
    s0 = EW[z].astype(ml_dtypes.bfloat16)                     # [N, 32] bf16

    # fused transform weights with norm constants folded in
    cs = C_SCALAR * np.float32(1.0 / np.sqrt(S_MUL))
    csb = C_SCALAR * np.float32(INV_SQRT3 / np.sqrt(S_MUL))
    cv = C_VECTOR * np.float32(INV_SQRT3 / np.sqrt(V_MUL))
    wta = np.zeros((3, 80, 80), np.float32)
    wtb = np.zeros((3, 112, 80), np.float32)
    wtc = np.zeros((3, 80, 80), np.float32)
    for l in range(3):
        Wa = cs * (W1[l] @ Ws[l])        # [32,32] s_m -> s
        Wb = csb * (W4[l] @ Ws[l])       # [16,32] v_c*sh_c -> s
        Wc = cv * (W2[l] @ Wv[l])        # [32,16] s*sh_c -> v_c
        Wd = cv * (W3[l] @ Wv[l])        # [16,16] v_c_m -> v_c
        # SSa rows: [s_m(0:32) vx_m(32:48) vy_m(48:64) vz_m(64:80)]
        wta[l, 0:32, 0:32] = Wa
        wta[l, 32:48, 32:48] = Wd
        wta[l, 48:64, 48:64] = Wd
        wta[l, 64:80, 64:80] = Wd
        # SSb rows: [s_y(0:32) waste(32:48) vyy(48:64) s_x(64:96) vxx(96:112)]
        wtb[l, 0:32, 48:64] = Wc
        wtb[l, 48:64, 0:32] = Wb
        wtb[l, 64:96, 32:48] = Wc
        wtb[l, 96:112, 0:32] = Wb
        # SSc rows: [s_z(0:32) waste(32:64) vzz(64:80)]
        wtc[l, 0:32, 64:80] = Wc
        wtc[l, 64:80, 0:32] = Wb

    # readout Wr1 with rows permuted to the [s | vx | vy | vz] feature order
    wr1p = np.zeros((80, HID), np.float32)
    wr1p[0:32] = Wr1[0:32]                        # s
    for u in range(V_MUL):
        wr1p[32 + u] = Wr1[S_MUL + 3 * u + 0]     # vx
        wr1p[48 + u] = Wr1[S_MUL + 3 * u + 1]     # vy
        wr1p[64 + u] = Wr1[S_MUL + 3 * u + 2]     # vz

    bdm = np.zeros((128, 128), np.float32)
    for g in range(GPB):
        bdm[g * NA:(g + 1) * NA, g * NA:(g + 1) * NA] = 1.0
    np.fill_diagonal(bdm, 0.0)                    # no self-loops (d2 > 0)
    poolm = np.zeros((128, GPB), np.float32)
    for g in range(GPB):
        poolm[g * NA:(g + 1) * NA, g] = 1.0

    if "nc" not in _CACHE:
        _CACHE["nc"] = _build_program()
    nc = _CACHE["nc"]

    in_maps = []
    for c in range(NCORES):
        psl = pos[c * NPC:(c + 1) * NPC]                       # [8192, 3]
        posT = np.ascontiguousarray(
            psl.reshape(NBLK, 128, 3).transpose(0, 2, 1))      # [64, 3, 128]
        in_maps.append(dict(
            s0=np.ascontiguousarray(s0[c * NPC:(c + 1) * NPC]),
            posT=posT,
            posnm=np.ascontiguousarray(psl),
            bd=bdm, wta=wta, wtb=wtb, wtc=wtc, poolm=poolm,
            wr1=wr1p, br1=br1.reshape(HID, 1),
            wr2=Wr2, br2=br2.reshape(LAT, 1),
        ))

    res = run_bass_kernel_spmd(nc, in_maps, core_ids=list(range(NCORES)))
    out = np.empty((B, LAT), np.float32)
    for c in range(NCORES):
        out[c * GPC:(c + 1) * GPC] = res.results[c]["outfm"].T
    return out

